# revision 17
# baseline (speedup 1.0000x reference)
"""CrossNet (DCN cross layers) forward on 8 Trainium2 NeuronCores.

Math: xl_{t+1} = x0 * (xl_t . w_t) + b_t + xl_t  stays in the affine span of
x0: xl_t = a_t * x0 + c_t with c_t = sum_{j<t} b_j a constant vector and a_t a
per-row scalar.  With u_t = x0 . w_t and g_t = c_t . w_t (weight-only consts):

    a_{t+1} = a_t * (1 + u_t) + g_t ,  a_0 = 1
    out     = a_L * x0 + sum_t b_t

So the whole network is 3 independent per-row dot products (u0,u1,u2), a tiny
scalar recurrence, and one scale-and-add -- one read of x, one write of out.

Sharding: data-parallel on batch, 2048 rows per core, weights replicated.

Engine split per [128,1024] row tile, mode "bf16" (DMA roofline is
~2.9us/tile/core; every engine is kept under it):

  GPSIMD SWDGE cast-load: x f32 in DRAM -> xb bf16 in SBUF (descriptor gen
         only; the cast rides the SDMA datapath)
  DVE    u0,u1,u2 via scalar_tensor_tensor in 2x_1p bf16 mode (fp32
         accum_out is a free-size-1 operand, so it keeps the 2x pricing),
         plus the tiny a3 recurrence (1x 1-port ops only)
  ACT    diag(a3) build (bf16), PSUM->SBUF copy
  PE     out_psum = diag(a3) @ xb  (bf16, 1 cyc/col)
                  + ones2 @ [dsum_hi; dsum_lo]  (bf16 K=2 rank-2 broadcast --
         dsum split into two bf16 parts keeps fp32-level accuracy)
  DMA    stores on sync HWDGE (separate ring from the SWDGE loads)

bf16 x and weights put the dot products and the a3*x0 product at ~1e-3
relative error -- far under the 2e-2 gate.  Mode "balanced" is the previous
all-fp32 split (8.4e-7) at ~2x the runtime.
"""

import os

import numpy as np

import concourse.mybir as mybir
from concourse.bacc import Bacc
from concourse.bass import Bass
from concourse.bass_utils import run_bass_kernel_spmd
from concourse.masks import make_identity
from concourse.tile import TileContext

B, D, L = 16384, 1024, 3
N_CORES = 8
RPC = B // N_CORES  # rows per core: 2048
P = 128
N_TILES = RPC // P  # 16
F32 = mybir.dt.float32
BF16 = mybir.dt.bfloat16

MODE = os.environ.get("CROSSNET_MODE", "bf16")
# "swdge": cast f32->bf16 during the load DMA (gpsimd descriptor path).
# "hwdge": plain f32 load on sync HWDGE + ACT convert to bf16.
LOAD = os.environ.get("CROSSNET_LOAD", "hwdge")
# "ttr": fused tensor_tensor_reduce dot. "stt": scalar_tensor_tensor+accum.
DOT = os.environ.get("CROSSNET_DOT", "ttr")
# Benchmark-only: repeat the whole body N times inside one program so
# per-iteration HW time can be extracted as a slope.
REPEAT = int(os.environ.get("CROSSNET_REPEAT", "1"))
# Benchmark-only: trace=True to pull an NTFF profile back through axon.
TRACE = os.environ.get("CROSSNET_TRACE", "0") == "1"
LAST_RESULTS = None

_CACHE: dict[str, Bass] = {}


def _build_bf16() -> Bass:
    nc = Bacc("TRN2", target_bir_lowering=False, debug=False, num_devices=N_CORES)
    x = nc.dram_tensor("x", [RPC, D], F32, kind="ExternalInput")
    # aux rows: 0..2 = w0,w1,w2 ; 3 = dsum (unused) ; 4 = [g1, g2]
    aux = nc.dram_tensor("aux", [5, D], F32, kind="ExternalInput")
    # dsum split into bf16 hi/lo rows: dsum ~= hi + lo to ~2^-16 relative.
    aux16 = nc.dram_tensor("aux16", [2, D], BF16, kind="ExternalInput")
    out = nc.dram_tensor("out", [RPC, D], F32, kind="ExternalOutput")

    mult = mybir.AluOpType.mult
    add = mybir.AluOpType.add
    Copy = mybir.ActivationFunctionType.Copy
    Ident = mybir.ActivationFunctionType.Identity

    with TileContext(nc) as tc:
        with (
            tc.tile_pool(name="consts", bufs=1) as consts,
            tc.tile_pool(name="xp", bufs=4) as xp,
            tc.tile_pool(name="scrp", bufs=2) as scrp,
            tc.tile_pool(name="op", bufs=4) as op,
            tc.tile_pool(name="small", bufs=8) as small,
            tc.tile_pool(name="diagp", bufs=3) as diagp,
            tc.tile_pool(name="psum", bufs=2, space="PSUM") as psum_pool,
            tc.tile_pool(name="psum_bc", bufs=2, space="PSUM") as psum_bc,
        ):
            # ---- one-time constants ----
            w_row = []
            for t in range(3):
                r = consts.tile([1, D], F32, tag=f"w_row{t}")
                nc.sync.dma_start(out=r, in_=aux[t : t + 1, :])
                w_row.append(r)
            g_row = consts.tile([1, 2], F32, tag="g_row")
            nc.sync.dma_start(out=g_row, in_=aux[4:5, 0:2])
            d16 = consts.tile([2, D], BF16, tag="d16")
            nc.sync.dma_start(out=d16, in_=aux16[:, :])

            ones_col = consts.tile([1, P], F32, tag="ones_col")
            nc.vector.memset(ones_col, 1.0)
            ones2_bf = consts.tile([2, P], BF16, tag="ones2_bf")
            nc.vector.memset(ones2_bf, 1.0)
            ident_bf = consts.tile([P, P], BF16, tag="ident_bf")
            make_identity(nc, ident_bf)

            # Broadcast w_t (cast to bf16) and [g1,g2] across partitions via
            # PE outer product: ones[1,P]^T @ row[1,N] -> [P, N] in PSUM.
            wbb = []
            for t in range(3):
                wt = consts.tile([P, D], BF16, tag=f"wbb{t}")
                for h in range(2):
                    sl = slice(512 * h, 512 * (h + 1))
                    ps = psum_bc.tile([P, 512], F32, tag="bc")
                    nc.tensor.matmul(ps, ones_col, w_row[t][:, sl], start=True, stop=True)
                    nc.scalar.copy(wt[:, sl], ps)
                wbb.append(wt)
            gb = consts.tile([P, 2], F32, tag="gb")
            ps = psum_bc.tile([P, 2], F32, tag="bc_g")
            nc.tensor.matmul(ps, ones_col, g_row[:, 0:2], start=True, stop=True)
            nc.scalar.copy(gb, ps)

            # ---- steady-state row tiles ----
            for i in range(N_TILES * REPEAT):
                i = i % N_TILES
                rows = slice(i * P, (i + 1) * P)
                xb = xp.tile([P, D], BF16, tag="xb")
                if LOAD == "swdge":
                    # SWDGE cast-load: f32 DRAM -> bf16 SBUF in the DMA
                    nc.gpsimd.dma_start(out=xb, in_=x[rows, :])
                else:
                    xt = xp.tile([P, D], F32, tag="xt")
                    nc.sync.dma_start(out=xt, in_=x[rows, :])
                    nc.scalar.copy(xb, xt)

                # v_t = 1 + xb.w_t in one fused DVE op per t: the reduce's
                # initial value (scalar=1.0) folds the +1 in.
                v = small.tile([P, 4], F32, tag="v")
                scr_b = scrp.tile([P, D], BF16, tag="scr_b")
                if DOT == "ttr":
                    for t in range(3):
                        nc.vector.tensor_tensor_reduce(
                            out=scr_b, in0=xb, in1=wbb[t], scale=1.0,
                            scalar=1.0, op0=mult, op1=add,
                            accum_out=v[:, t : t + 1],
                        )
                else:
                    # accum starts at 0 -> add 1 on ACT afterwards
                    u = small.tile([P, 4], F32, tag="u")
                    for t in range(3):
                        nc.vector.scalar_tensor_tensor(
                            out=scr_b, in0=xb, scalar=1.0, in1=wbb[t],
                            op0=mult, op1=mult, accum_out=u[:, t : t + 1],
                        )
                    nc.scalar.activation(v[:, 0:3], u[:, 0:3], Ident, bias=1.0)

                # recurrence: a2 = v0*v1+g1 ; a3 = a2*v2+g2 on ACT (tiny DVE
                # ops cost ~475ns each in SBUF access latency; ACT has
                # headroom and does them at ~300ns)
                a2 = small.tile([P, 1], F32, tag="a2")
                a3 = small.tile([P, 1], F32, tag="a3")
                nc.scalar.activation(
                    a2, v[:, 0:1], Ident, scale=v[:, 1:2], bias=gb[:, 0:1]
                )
                nc.scalar.activation(
                    a3, a2, Ident, scale=v[:, 2:3], bias=gb[:, 1:2]
                )
                # diag(a3) on DVE in bf16 2x: (ident*a3)*ident == diag(a3)
                # (off-diagonal entries are 0 either way)
                diag = diagp.tile([P, P], BF16, tag="diag")
                nc.vector.scalar_tensor_tensor(
                    out=diag, in0=ident_bf, scalar=a3[:, 0:1], in1=ident_bf,
                    op0=mult, op1=mult,
                )

                # out_psum = diag(a3) @ xb + ones2 @ [dsum_hi; dsum_lo]
                ps_out = psum_pool.tile([P, D], F32, tag="ps_out")
                for h in range(2):
                    sl = slice(512 * h, 512 * (h + 1))
                    nc.tensor.matmul(
                        ps_out[:, sl], diag, xb[:, sl], start=True, stop=False
                    )
                for h in range(2):
                    sl = slice(512 * h, 512 * (h + 1))
                    nc.tensor.matmul(
                        ps_out[:, sl], ones2_bf, d16[:, sl], start=False, stop=True
                    )

                ot = op.tile([P, D], F32, tag="ot")
                nc.scalar.copy(ot, ps_out)
                nc.sync.dma_start(out=out[rows, :], in_=ot)

    nc.compile()
    return nc


def _build_balanced() -> Bass:
    # Previous all-fp32 engine split (rel err 8.4e-7, ~2x slower): DVE does
    # u0,u1 in fp32 1x, GPSIMD multiplies for u2, ACT reduces, PE applies
    # diag(a3) in fp32 + bf16 dsum broadcast.
    nc = Bacc("TRN2", target_bir_lowering=False, debug=False, num_devices=N_CORES)
    x = nc.dram_tensor("x", [RPC, D], F32, kind="ExternalInput")
    aux = nc.dram_tensor("aux", [5, D], F32, kind="ExternalInput")
    aux16 = nc.dram_tensor("aux16", [2, D], BF16, kind="ExternalInput")
    out = nc.dram_tensor("out", [RPC, D], F32, kind="ExternalOutput")

    mult = mybir.AluOpType.mult
    add = mybir.AluOpType.add
    Copy = mybir.ActivationFunctionType.Copy

    with TileContext(nc) as tc:
        with (
            tc.tile_pool(name="consts", bufs=1) as consts,
            tc.tile_pool(name="xp", bufs=6) as xp,
            tc.tile_pool(name="scrp", bufs=3) as scrp,
            tc.tile_pool(name="op", bufs=4) as op,
            tc.tile_pool(name="small", bufs=8) as small,
            tc.tile_pool(name="diagp", bufs=3) as diagp,
            tc.tile_pool(name="psum", bufs=2, space="PSUM") as psum_pool,
            tc.tile_pool(name="psum_bc", bufs=2, space="PSUM") as psum_bc,
        ):
            w_row = []
            for t in range(3):
                r = consts.tile([1, D], F32, tag=f"w_row{t}")
                nc.sync.dma_start(out=r, in_=aux[t : t + 1, :])
                w_row.append(r)
            g_row = consts.tile([1, 2], F32, tag="g_row")
            nc.sync.dma_start(out=g_row, in_=aux[4:5, 0:2])
            d16 = consts.tile([2, D], BF16, tag="d16")
            nc.sync.dma_start(out=d16, in_=aux16[:, :])

            ones_col = consts.tile([1, P], F32, tag="ones_col")
            nc.vector.memset(ones_col, 1.0)
            ones2_bf = consts.tile([2, P], BF16, tag="ones2_bf")
            nc.vector.memset(ones2_bf, 1.0)
            ones4 = consts.tile([P, 4], F32, tag="ones4")
            nc.vector.memset(ones4, 1.0)
            ident = consts.tile([P, P], F32, tag="ident")
            make_identity(nc, ident)

            wb = []
            for t in range(3):
                wt = consts.tile([P, D], F32, tag=f"wb{t}")
                for h in range(2):
                    sl = slice(512 * h, 512 * (h + 1))
                    ps = psum_bc.tile([P, 512], F32, tag="bc")
                    nc.tensor.matmul(ps, ones_col, w_row[t][:, sl], start=True, stop=True)
                    nc.scalar.copy(wt[:, sl], ps)
                wb.append(wt)
            gb = consts.tile([P, 2], F32, tag="gb")
            ps = psum_bc.tile([P, 2], F32, tag="bc_g")
            nc.tensor.matmul(ps, ones_col, g_row[:, 0:2], start=True, stop=True)
            nc.scalar.copy(gb, ps)

            for i in range(N_TILES * REPEAT):
                i = i % N_TILES
                rows = slice(i * P, (i + 1) * P)
                xt = xp.tile([P, D], F32, tag="x")
                nc.sync.dma_start(out=xt, in_=x[rows, :])

                u = small.tile([P, 4], F32, tag="u")
                scr = small.tile([P, D], F32, tag="scr")

                scr2 = scrp.tile([P, D], F32, tag="scr2")
                nc.gpsimd.tensor_tensor(scr2, xt, wb[2], op=mult)
                dummy2 = small.tile([P, 1], F32, tag="dummy2")
                nc.scalar.activation(
                    dummy2.broadcast_to((P, D)), scr2, Copy,
                    accum_out=u[:, 2:3],
                )
                for t in range(2):
                    nc.vector.scalar_tensor_tensor(
                        out=scr, in0=xt, scalar=1.0, in1=wb[t],
                        op0=mult, op1=mult, accum_out=u[:, t : t + 1],
                    )

                v = small.tile([P, 3], F32, tag="v")
                a2 = small.tile([P, 1], F32, tag="a2")
                a3 = small.tile([P, 1], F32, tag="a3")
                nc.vector.tensor_tensor(v, u[:, 0:3], ones4[:, 0:3], op=add)
                nc.vector.scalar_tensor_tensor(
                    out=a2, in0=v[:, 0:1], scalar=v[:, 1:2], in1=gb[:, 0:1],
                    op0=mult, op1=add,
                )
                nc.vector.scalar_tensor_tensor(
                    out=a3, in0=a2, scalar=v[:, 2:3], in1=gb[:, 1:2],
                    op0=mult, op1=add,
                )
                diag = diagp.tile([P, P], F32, tag="diag")
                nc.scalar.activation(diag, ident, Copy, scale=a3[:, 0:1])

                ps_out = psum_pool.tile([P, D], F32, tag="ps_out")
                for h in range(2):
                    sl = slice(512 * h, 512 * (h + 1))
                    nc.tensor.matmul(
                        ps_out[:, sl], diag, xt[:, sl], start=True, stop=False
                    )
                for h in range(2):
                    sl = slice(512 * h, 512 * (h + 1))
                    nc.tensor.matmul(
                        ps_out[:, sl], ones2_bf, d16[:, sl], start=False, stop=True
                    )

                ot = op.tile([P, D], F32, tag="ot")
                nc.scalar.copy(ot, ps_out)
                nc.sync.dma_start(out=out[rows, :], in_=ot)

    nc.compile()
    return nc


def _build() -> Bass:
    return _build_bf16() if MODE == "bf16" else _build_balanced()


def _get_program() -> Bass:
    key = f"{MODE}-{LOAD}-{DOT}-{REPEAT}"
    if key not in _CACHE:
        _CACHE[key] = _build()
    return _CACHE[key]


def _make_aux(weights: np.ndarray, bias: np.ndarray):
    import ml_dtypes

    w = np.asarray(weights, dtype=np.float32)
    b = np.asarray(bias, dtype=np.float32)
    aux = np.zeros((5, D), dtype=np.float32)
    aux[0:3] = w
    dsum = b.sum(axis=0)
    aux[3] = dsum
    aux[4, 0] = float(b[0] @ w[1])
    aux[4, 1] = float((b[0] + b[1]) @ w[2])
    hi = dsum.astype(ml_dtypes.bfloat16)
    lo = (dsum - hi.astype(np.float32)).astype(ml_dtypes.bfloat16)
    aux16 = np.stack([hi, lo])
    return aux, aux16


def kernel(x: np.ndarray, weights: np.ndarray, bias: np.ndarray) -> np.ndarray:
    x = np.ascontiguousarray(np.asarray(x, dtype=np.float32))
    aux, aux16 = _make_aux(weights, bias)
    nc = _get_program()
    in_maps = [
        {"x": x[i * RPC : (i + 1) * RPC], "aux": aux, "aux16": aux16}
        for i in range(N_CORES)
    ]
    res = run_bass_kernel_spmd(nc, in_maps, list(range(N_CORES)), trace=TRACE)
    global LAST_RESULTS
    LAST_RESULTS = res
    return np.concatenate([r["out"] for r in res.results], axis=0)


# revision 20
# speedup vs baseline: 1.0016x; 1.0016x over previous
"""CrossNet (DCN cross layers) forward on 8 Trainium2 NeuronCores.

Math: xl_{t+1} = x0 * (xl_t . w_t) + b_t + xl_t  stays in the affine span of
x0: xl_t = a_t * x0 + c_t with c_t = sum_{j<t} b_j a constant vector and a_t a
per-row scalar.  With u_t = x0 . w_t and g_t = c_t . w_t (weight-only consts):

    a_{t+1} = a_t * (1 + u_t) + g_t ,  a_0 = 1
    out     = a_L * x0 + sum_t b_t

So the whole network is 3 independent per-row dot products (u0,u1,u2), a tiny
scalar recurrence, and one scale-and-add -- one read of x, one write of out.

Sharding: data-parallel on batch, 2048 rows per core, weights replicated.

Engine split per [128,1024] row tile, mode "bf16" (DMA roofline is
~2.9us/tile/core; every engine is kept under it):

  GPSIMD SWDGE cast-load: x f32 in DRAM -> xb bf16 in SBUF (descriptor gen
         only; the cast rides the SDMA datapath)
  DVE    u0,u1,u2 via scalar_tensor_tensor in 2x_1p bf16 mode (fp32
         accum_out is a free-size-1 operand, so it keeps the 2x pricing),
         plus the tiny a3 recurrence (1x 1-port ops only)
  ACT    diag(a3) build (bf16), PSUM->SBUF copy
  PE     out_psum = diag(a3) @ xb  (bf16, 1 cyc/col)
                  + ones2 @ [dsum_hi; dsum_lo]  (bf16 K=2 rank-2 broadcast --
         dsum split into two bf16 parts keeps fp32-level accuracy)
  DMA    stores on sync HWDGE (separate ring from the SWDGE loads)

bf16 x and weights put the dot products and the a3*x0 product at ~1e-3
relative error -- far under the 2e-2 gate.  Mode "balanced" is the previous
all-fp32 split (8.4e-7) at ~2x the runtime.
"""

import os

import numpy as np

import concourse.mybir as mybir
from concourse.bacc import Bacc
from concourse.bass import Bass
from concourse.bass_utils import run_bass_kernel_spmd
from concourse.masks import make_identity
from concourse.tile import TileContext

B, D, L = 16384, 1024, 3
N_CORES = 8
RPC = B // N_CORES  # rows per core: 2048
P = 128
N_TILES = RPC // P  # 16
F32 = mybir.dt.float32
BF16 = mybir.dt.bfloat16

MODE = os.environ.get("CROSSNET_MODE", "bf16")
# "swdge": cast f32->bf16 during the load DMA (gpsimd descriptor path).
# "hwdge": plain f32 load on sync HWDGE + ACT convert to bf16.
LOAD = os.environ.get("CROSSNET_LOAD", "swdge")
# "ttr": fused tensor_tensor_reduce dot (breaks on HW!). "stt": stt+accum.
DOT = os.environ.get("CROSSNET_DOT", "stt")
# "gpsimd": third dot entirely on GPSIMD (stt+accum). "dve": all three on DVE.
DOTC = os.environ.get("CROSSNET_DOTC", "gpsimd")
# Benchmark-only: repeat the whole body N times inside one program so
# per-iteration HW time can be extracted as a slope.
REPEAT = int(os.environ.get("CROSSNET_REPEAT", "1"))
# Benchmark-only: trace=True to pull an NTFF profile back through axon.
TRACE = os.environ.get("CROSSNET_TRACE", "0") == "1"
LAST_RESULTS = None

_CACHE: dict[str, Bass] = {}


def _build_bf16() -> Bass:
    nc = Bacc("TRN2", target_bir_lowering=False, debug=False, num_devices=N_CORES)
    x = nc.dram_tensor("x", [RPC, D], F32, kind="ExternalInput")
    # aux rows: 0..2 = w0,w1,w2 ; 3 = dsum (unused) ; 4 = [g1, g2]
    aux = nc.dram_tensor("aux", [5, D], F32, kind="ExternalInput")
    # dsum split into bf16 hi/lo rows: dsum ~= hi + lo to ~2^-16 relative.
    aux16 = nc.dram_tensor("aux16", [2, D], BF16, kind="ExternalInput")
    out = nc.dram_tensor("out", [RPC, D], F32, kind="ExternalOutput")

    mult = mybir.AluOpType.mult
    add = mybir.AluOpType.add
    Copy = mybir.ActivationFunctionType.Copy
    Ident = mybir.ActivationFunctionType.Identity

    with TileContext(nc) as tc:
        with (
            tc.tile_pool(name="consts", bufs=1) as consts,
            tc.tile_pool(name="xp", bufs=4) as xp,
            tc.tile_pool(name="scrp", bufs=2) as scrp,
            tc.tile_pool(name="op", bufs=4) as op,
            tc.tile_pool(name="small", bufs=8) as small,
            tc.tile_pool(name="diagp", bufs=3) as diagp,
            tc.tile_pool(name="psum", bufs=2, space="PSUM") as psum_pool,
            tc.tile_pool(name="psum_bc", bufs=2, space="PSUM") as psum_bc,
        ):
            # ---- one-time constants ----
            w_row = []
            for t in range(3):
                r = consts.tile([1, D], F32, tag=f"w_row{t}")
                nc.sync.dma_start(out=r, in_=aux[t : t + 1, :])
                w_row.append(r)
            g_row = consts.tile([1, 2], F32, tag="g_row")
            nc.sync.dma_start(out=g_row, in_=aux[4:5, 0:2])
            d16 = consts.tile([2, D], BF16, tag="d16")
            nc.sync.dma_start(out=d16, in_=aux16[:, :])

            ones_col = consts.tile([1, P], F32, tag="ones_col")
            nc.vector.memset(ones_col, 1.0)
            ones2_bf = consts.tile([2, P], BF16, tag="ones2_bf")
            nc.vector.memset(ones2_bf, 1.0)
            ident_bf = consts.tile([P, P], BF16, tag="ident_bf")
            make_identity(nc, ident_bf)

            # Broadcast w_t (cast to bf16) and [g1,g2] across partitions via
            # PE outer product: ones[1,P]^T @ row[1,N] -> [P, N] in PSUM.
            wbb = []
            for t in range(3):
                wt = consts.tile([P, D], BF16, tag=f"wbb{t}")
                for h in range(2):
                    sl = slice(512 * h, 512 * (h + 1))
                    ps = psum_bc.tile([P, 512], F32, tag="bc")
                    nc.tensor.matmul(ps, ones_col, w_row[t][:, sl], start=True, stop=True)
                    nc.scalar.copy(wt[:, sl], ps)
                wbb.append(wt)
            gb = consts.tile([P, 2], F32, tag="gb")
            ps = psum_bc.tile([P, 2], F32, tag="bc_g")
            nc.tensor.matmul(ps, ones_col, g_row[:, 0:2], start=True, stop=True)
            nc.scalar.copy(gb, ps)

            # ---- steady-state row tiles, lag-1 software pipeline ----
            # front half (tile i): load + 3 dot products
            # back half (tile i-1): recurrence + diag + PE out + store
            # Emitting back(i-1) after front(i) keeps diag(i-1) from
            # blocking the DVE queue while ACT runs the a3 chain, and
            # keeps every engine's FIFO free of cross-engine waits.
            def front(i):
                rows = slice(i * P, (i + 1) * P)
                xb = xp.tile([P, D], BF16, tag="xb")
                if LOAD == "swdge":
                    # SWDGE cast-load: f32 DRAM -> bf16 SBUF in the DMA
                    nc.gpsimd.dma_start(out=xb, in_=x[rows, :])
                else:
                    xt = xp.tile([P, D], F32, tag="xt")
                    nc.sync.dma_start(out=xt, in_=x[rows, :])
                    nc.scalar.copy(xb, xt)

                u = small.tile([P, 4], F32, tag="u")
                scr_b = scrp.tile([P, D], BF16, tag="scr_b")
                ndve = 2 if DOTC == "gpsimd" else 3
                for t in range(ndve):
                    nc.vector.scalar_tensor_tensor(
                        out=scr_b, in0=xb, scalar=1.0, in1=wbb[t],
                        op0=mult, op1=mult, accum_out=u[:, t : t + 1],
                    )
                if DOTC == "gpsimd":
                    # STT is not a valid Pool opcode: multiply on GPSIMD,
                    # accum-reduce on ACT (dummy broadcast output)
                    scr_c = scrp.tile([P, D], BF16, tag="scr_c")
                    nc.gpsimd.tensor_tensor(scr_c, xb, wbb[2], op=mult)
                    dummy = small.tile([P, 1], F32, tag="dummy")
                    nc.scalar.activation(
                        dummy.broadcast_to((P, D)), scr_c, Copy,
                        accum_out=u[:, 2:3],
                    )
                return rows, xb, u

            def back(state):
                rows, xb, u = state
                # v = 1+u ; a2 = v0*v1+g1 ; a3 = a2*v2+g2 on ACT
                v = small.tile([P, 3], F32, tag="v")
                a2 = small.tile([P, 1], F32, tag="a2")
                a3 = small.tile([P, 1], F32, tag="a3")
                nc.scalar.activation(v, u[:, 0:3], Ident, bias=1.0)
                nc.scalar.activation(
                    a2, v[:, 0:1], Ident, scale=v[:, 1:2], bias=gb[:, 0:1]
                )
                nc.scalar.activation(
                    a3, a2, Ident, scale=v[:, 2:3], bias=gb[:, 1:2]
                )
                # diag(a3) on DVE in bf16: (ident*a3)*ident == diag(a3)
                diag = diagp.tile([P, P], BF16, tag="diag")
                nc.vector.scalar_tensor_tensor(
                    out=diag, in0=ident_bf, scalar=a3[:, 0:1], in1=ident_bf,
                    op0=mult, op1=mult,
                )

                # out_psum = diag(a3) @ xb + ones2 @ [dsum_hi; dsum_lo]
                ps_out = psum_pool.tile([P, D], F32, tag="ps_out")
                for h in range(2):
                    sl = slice(512 * h, 512 * (h + 1))
                    nc.tensor.matmul(
                        ps_out[:, sl], diag, xb[:, sl], start=True, stop=False
                    )
                for h in range(2):
                    sl = slice(512 * h, 512 * (h + 1))
                    nc.tensor.matmul(
                        ps_out[:, sl], ones2_bf, d16[:, sl], start=False, stop=True
                    )

                ot = op.tile([P, D], F32, tag="ot")
                nc.scalar.copy(ot, ps_out)
                nc.sync.dma_start(out=out[rows, :], in_=ot)

            pending = None
            for i in range(N_TILES * REPEAT):
                state = front(i % N_TILES)
                if pending is not None:
                    back(pending)
                pending = state
            back(pending)

    nc.compile()
    return nc


def _build_balanced() -> Bass:
    # Previous all-fp32 engine split (rel err 8.4e-7, ~2x slower): DVE does
    # u0,u1 in fp32 1x, GPSIMD multiplies for u2, ACT reduces, PE applies
    # diag(a3) in fp32 + bf16 dsum broadcast.
    nc = Bacc("TRN2", target_bir_lowering=False, debug=False, num_devices=N_CORES)
    x = nc.dram_tensor("x", [RPC, D], F32, kind="ExternalInput")
    aux = nc.dram_tensor("aux", [5, D], F32, kind="ExternalInput")
    aux16 = nc.dram_tensor("aux16", [2, D], BF16, kind="ExternalInput")
    out = nc.dram_tensor("out", [RPC, D], F32, kind="ExternalOutput")

    mult = mybir.AluOpType.mult
    add = mybir.AluOpType.add
    Copy = mybir.ActivationFunctionType.Copy

    with TileContext(nc) as tc:
        with (
            tc.tile_pool(name="consts", bufs=1) as consts,
            tc.tile_pool(name="xp", bufs=6) as xp,
            tc.tile_pool(name="scrp", bufs=3) as scrp,
            tc.tile_pool(name="op", bufs=4) as op,
            tc.tile_pool(name="small", bufs=8) as small,
            tc.tile_pool(name="diagp", bufs=3) as diagp,
            tc.tile_pool(name="psum", bufs=2, space="PSUM") as psum_pool,
            tc.tile_pool(name="psum_bc", bufs=2, space="PSUM") as psum_bc,
        ):
            w_row = []
            for t in range(3):
                r = consts.tile([1, D], F32, tag=f"w_row{t}")
                nc.sync.dma_start(out=r, in_=aux[t : t + 1, :])
                w_row.append(r)
            g_row = consts.tile([1, 2], F32, tag="g_row")
            nc.sync.dma_start(out=g_row, in_=aux[4:5, 0:2])
            d16 = consts.tile([2, D], BF16, tag="d16")
            nc.sync.dma_start(out=d16, in_=aux16[:, :])

            ones_col = consts.tile([1, P], F32, tag="ones_col")
            nc.vector.memset(ones_col, 1.0)
            ones2_bf = consts.tile([2, P], BF16, tag="ones2_bf")
            nc.vector.memset(ones2_bf, 1.0)
            ones4 = consts.tile([P, 4], F32, tag="ones4")
            nc.vector.memset(ones4, 1.0)
            ident = consts.tile([P, P], F32, tag="ident")
            make_identity(nc, ident)

            wb = []
            for t in range(3):
                wt = consts.tile([P, D], F32, tag=f"wb{t}")
                for h in range(2):
                    sl = slice(512 * h, 512 * (h + 1))
                    ps = psum_bc.tile([P, 512], F32, tag="bc")
                    nc.tensor.matmul(ps, ones_col, w_row[t][:, sl], start=True, stop=True)
                    nc.scalar.copy(wt[:, sl], ps)
                wb.append(wt)
            gb = consts.tile([P, 2], F32, tag="gb")
            ps = psum_bc.tile([P, 2], F32, tag="bc_g")
            nc.tensor.matmul(ps, ones_col, g_row[:, 0:2], start=True, stop=True)
            nc.scalar.copy(gb, ps)

            for i in range(N_TILES * REPEAT):
                i = i % N_TILES
                rows = slice(i * P, (i + 1) * P)
                xt = xp.tile([P, D], F32, tag="x")
                nc.sync.dma_start(out=xt, in_=x[rows, :])

                u = small.tile([P, 4], F32, tag="u")
                scr = small.tile([P, D], F32, tag="scr")

                scr2 = scrp.tile([P, D], F32, tag="scr2")
                nc.gpsimd.tensor_tensor(scr2, xt, wb[2], op=mult)
                dummy2 = small.tile([P, 1], F32, tag="dummy2")
                nc.scalar.activation(
                    dummy2.broadcast_to((P, D)), scr2, Copy,
                    accum_out=u[:, 2:3],
                )
                for t in range(2):
                    nc.vector.scalar_tensor_tensor(
                        out=scr, in0=xt, scalar=1.0, in1=wb[t],
                        op0=mult, op1=mult, accum_out=u[:, t : t + 1],
                    )

                v = small.tile([P, 3], F32, tag="v")
                a2 = small.tile([P, 1], F32, tag="a2")
                a3 = small.tile([P, 1], F32, tag="a3")
                nc.vector.tensor_tensor(v, u[:, 0:3], ones4[:, 0:3], op=add)
                nc.vector.scalar_tensor_tensor(
                    out=a2, in0=v[:, 0:1], scalar=v[:, 1:2], in1=gb[:, 0:1],
                    op0=mult, op1=add,
                )
                nc.vector.scalar_tensor_tensor(
                    out=a3, in0=a2, scalar=v[:, 2:3], in1=gb[:, 1:2],
                    op0=mult, op1=add,
                )
                diag = diagp.tile([P, P], F32, tag="diag")
                nc.scalar.activation(diag, ident, Copy, scale=a3[:, 0:1])

                ps_out = psum_pool.tile([P, D], F32, tag="ps_out")
                for h in range(2):
                    sl = slice(512 * h, 512 * (h + 1))
                    nc.tensor.matmul(
                        ps_out[:, sl], diag, xt[:, sl], start=True, stop=False
                    )
                for h in range(2):
                    sl = slice(512 * h, 512 * (h + 1))
                    nc.tensor.matmul(
                        ps_out[:, sl], ones2_bf, d16[:, sl], start=False, stop=True
                    )

                ot = op.tile([P, D], F32, tag="ot")
                nc.scalar.copy(ot, ps_out)
                nc.sync.dma_start(out=out[rows, :], in_=ot)

    nc.compile()
    return nc


def _build() -> Bass:
    return _build_bf16() if MODE == "bf16" else _build_balanced()


def _get_program() -> Bass:
    key = f"{MODE}-{LOAD}-{DOT}-{REPEAT}"
    if key not in _CACHE:
        _CACHE[key] = _build()
    return _CACHE[key]


def _make_aux(weights: np.ndarray, bias: np.ndarray):
    import ml_dtypes

    w = np.asarray(weights, dtype=np.float32)
    b = np.asarray(bias, dtype=np.float32)
    aux = np.zeros((5, D), dtype=np.float32)
    aux[0:3] = w
    dsum = b.sum(axis=0)
    aux[3] = dsum
    aux[4, 0] = float(b[0] @ w[1])
    aux[4, 1] = float((b[0] + b[1]) @ w[2])
    hi = dsum.astype(ml_dtypes.bfloat16)
    lo = (dsum - hi.astype(np.float32)).astype(ml_dtypes.bfloat16)
    aux16 = np.stack([hi, lo])
    return aux, aux16


def kernel(x: np.ndarray, weights: np.ndarray, bias: np.ndarray) -> np.ndarray:
    x = np.ascontiguousarray(np.asarray(x, dtype=np.float32))
    aux, aux16 = _make_aux(weights, bias)
    nc = _get_program()
    in_maps = [
        {"x": x[i * RPC : (i + 1) * RPC], "aux": aux, "aux16": aux16}
        for i in range(N_CORES)
    ]
    res = run_bass_kernel_spmd(nc, in_maps, list(range(N_CORES)), trace=TRACE)
    global LAST_RESULTS
    LAST_RESULTS = res
    return np.concatenate([r["out"] for r in res.results], axis=0)


# revision 21
# speedup vs baseline: 1.0070x; 1.0054x over previous
"""CrossNet (DCN cross layers) forward on 8 Trainium2 NeuronCores.

Math: xl_{t+1} = x0 * (xl_t . w_t) + b_t + xl_t  stays in the affine span of
x0: xl_t = a_t * x0 + c_t with c_t = sum_{j<t} b_j a constant vector and a_t a
per-row scalar.  With u_t = x0 . w_t and g_t = c_t . w_t (weight-only consts):

    a_{t+1} = a_t * (1 + u_t) + g_t ,  a_0 = 1
    out     = a_L * x0 + sum_t b_t

So the whole network is 3 independent per-row dot products (u0,u1,u2), a tiny
scalar recurrence, and one scale-and-add -- one read of x, one write of out.

Sharding: data-parallel on batch, 2048 rows per core, weights replicated.

Engine split per [128,1024] row tile, mode "bf16" (DMA roofline is
~2.9us/tile/core; every engine is kept under it):

  GPSIMD SWDGE cast-load: x f32 in DRAM -> xb bf16 in SBUF (descriptor gen
         only; the cast rides the SDMA datapath)
  DVE    u0,u1,u2 via scalar_tensor_tensor in 2x_1p bf16 mode (fp32
         accum_out is a free-size-1 operand, so it keeps the 2x pricing),
         plus the tiny a3 recurrence (1x 1-port ops only)
  ACT    diag(a3) build (bf16), PSUM->SBUF copy
  PE     out_psum = diag(a3) @ xb  (bf16, 1 cyc/col)
                  + ones2 @ [dsum_hi; dsum_lo]  (bf16 K=2 rank-2 broadcast --
         dsum split into two bf16 parts keeps fp32-level accuracy)
  DMA    stores on sync HWDGE (separate ring from the SWDGE loads)

bf16 x and weights put the dot products and the a3*x0 product at ~1e-3
relative error -- far under the 2e-2 gate.  Mode "balanced" is the previous
all-fp32 split (8.4e-7) at ~2x the runtime.
"""

import os

import numpy as np

import concourse.mybir as mybir
from concourse.bacc import Bacc
from concourse.bass import Bass
from concourse.bass_utils import run_bass_kernel_spmd
from concourse.masks import make_identity
from concourse.tile import TileContext

B, D, L = 16384, 1024, 3
N_CORES = 8
RPC = B // N_CORES  # rows per core: 2048
P = 128
N_TILES = RPC // P  # 16
F32 = mybir.dt.float32
BF16 = mybir.dt.bfloat16

MODE = os.environ.get("CROSSNET_MODE", "bf16")
# "swdge": cast f32->bf16 during the load DMA (gpsimd descriptor path).
# "hwdge": plain f32 load on sync HWDGE + ACT convert to bf16.
LOAD = os.environ.get("CROSSNET_LOAD", "swdge")
# "ttr": fused tensor_tensor_reduce dot (breaks on HW!). "stt": stt+accum.
DOT = os.environ.get("CROSSNET_DOT", "stt")
# "gpsimd": third dot entirely on GPSIMD (stt+accum). "dve": all three on DVE.
DOTC = os.environ.get("CROSSNET_DOTC", "gpsimd")
# compute dtype for x / weights / diag: "f32" skips the bf16 convert pass.
CDT = os.environ.get("CROSSNET_CDT", "f32")
# Benchmark-only: repeat the whole body N times inside one program so
# per-iteration HW time can be extracted as a slope.
REPEAT = int(os.environ.get("CROSSNET_REPEAT", "1"))
# Benchmark-only: trace=True to pull an NTFF profile back through axon.
TRACE = os.environ.get("CROSSNET_TRACE", "0") == "1"
LAST_RESULTS = None

_CACHE: dict[str, Bass] = {}


def _build_bf16() -> Bass:
    nc = Bacc("TRN2", target_bir_lowering=False, debug=False, num_devices=N_CORES)
    x = nc.dram_tensor("x", [RPC, D], F32, kind="ExternalInput")
    # aux rows: 0..2 = w0,w1,w2 ; 3 = dsum (unused) ; 4 = [g1, g2]
    aux = nc.dram_tensor("aux", [5, D], F32, kind="ExternalInput")
    # dsum split into bf16 hi/lo rows: dsum ~= hi + lo to ~2^-16 relative.
    aux16 = nc.dram_tensor("aux16", [2, D], BF16, kind="ExternalInput")
    out = nc.dram_tensor("out", [RPC, D], F32, kind="ExternalOutput")

    mult = mybir.AluOpType.mult
    add = mybir.AluOpType.add
    Copy = mybir.ActivationFunctionType.Copy
    Ident = mybir.ActivationFunctionType.Identity
    CD = BF16 if CDT == "bf16" else F32

    with TileContext(nc) as tc:
        with (
            tc.tile_pool(name="consts", bufs=1) as consts,
            tc.tile_pool(name="xp", bufs=4) as xp,
            tc.tile_pool(name="scrp", bufs=2) as scrp,
            tc.tile_pool(name="op", bufs=4) as op,
            tc.tile_pool(name="small", bufs=8) as small,
            tc.tile_pool(name="diagp", bufs=3) as diagp,
            tc.tile_pool(name="psum", bufs=2, space="PSUM") as psum_pool,
            tc.tile_pool(name="psum_bc", bufs=2, space="PSUM") as psum_bc,
        ):
            # ---- one-time constants ----
            w_row = []
            for t in range(3):
                r = consts.tile([1, D], F32, tag=f"w_row{t}")
                nc.sync.dma_start(out=r, in_=aux[t : t + 1, :])
                w_row.append(r)
            g_row = consts.tile([1, 2], F32, tag="g_row")
            nc.sync.dma_start(out=g_row, in_=aux[4:5, 0:2])
            d16 = consts.tile([2, D], BF16, tag="d16")
            nc.sync.dma_start(out=d16, in_=aux16[:, :])

            ones_col = consts.tile([1, P], F32, tag="ones_col")
            nc.vector.memset(ones_col, 1.0)
            ones2_bf = consts.tile([2, P], BF16, tag="ones2_bf")
            nc.vector.memset(ones2_bf, 1.0)
            ident_bf = consts.tile([P, P], CD, tag="ident_bf")
            make_identity(nc, ident_bf)

            # Broadcast w_t (cast to bf16) and [g1,g2] across partitions via
            # PE outer product: ones[1,P]^T @ row[1,N] -> [P, N] in PSUM.
            wbb = []
            for t in range(3):
                wt = consts.tile([P, D], CD, tag=f"wbb{t}")
                for h in range(2):
                    sl = slice(512 * h, 512 * (h + 1))
                    ps = psum_bc.tile([P, 512], F32, tag="bc")
                    nc.tensor.matmul(ps, ones_col, w_row[t][:, sl], start=True, stop=True)
                    nc.scalar.copy(wt[:, sl], ps)
                wbb.append(wt)
            gb = consts.tile([P, 2], F32, tag="gb")
            ps = psum_bc.tile([P, 2], F32, tag="bc_g")
            nc.tensor.matmul(ps, ones_col, g_row[:, 0:2], start=True, stop=True)
            nc.scalar.copy(gb, ps)

            # ---- steady-state row tiles, lag-1 software pipeline ----
            # front half (tile i): load + 3 dot products
            # back half (tile i-1): recurrence + diag + PE out + store
            # Emitting back(i-1) after front(i) keeps diag(i-1) from
            # blocking the DVE queue while ACT runs the a3 chain, and
            # keeps every engine's FIFO free of cross-engine waits.
            def front(i):
                rows = slice(i * P, (i + 1) * P)
                xb = xp.tile([P, D], CD, tag="xb")
                if LOAD == "swdge" and CD is BF16:
                    # SWDGE cast-load: f32 DRAM -> bf16 SBUF in the DMA
                    nc.gpsimd.dma_start(out=xb, in_=x[rows, :])
                elif CD is BF16:
                    xt = xp.tile([P, D], F32, tag="xt")
                    nc.sync.dma_start(out=xt, in_=x[rows, :])
                    nc.scalar.copy(xb, xt)
                else:
                    nc.sync.dma_start(out=xb, in_=x[rows, :])

                u = small.tile([P, 4], F32, tag="u")
                scr_b = scrp.tile([P, D], CD, tag="scr_b")
                ndve = 2 if DOTC == "gpsimd" else 3
                for t in range(ndve):
                    nc.vector.scalar_tensor_tensor(
                        out=scr_b, in0=xb, scalar=1.0, in1=wbb[t],
                        op0=mult, op1=mult, accum_out=u[:, t : t + 1],
                    )
                if DOTC == "gpsimd":
                    # STT is not a valid Pool opcode: multiply on GPSIMD,
                    # accum-reduce on ACT (dummy broadcast output)
                    scr_c = scrp.tile([P, D], CD, tag="scr_c")
                    nc.gpsimd.tensor_tensor(scr_c, xb, wbb[2], op=mult)
                    dummy = small.tile([P, 1], F32, tag="dummy")
                    nc.scalar.activation(
                        dummy.broadcast_to((P, D)), scr_c, Copy,
                        accum_out=u[:, 2:3],
                    )
                return rows, xb, u

            def back(state):
                rows, xb, u = state
                # v = 1+u ; a2 = v0*v1+g1 ; a3 = a2*v2+g2 on ACT
                v = small.tile([P, 3], F32, tag="v")
                a2 = small.tile([P, 1], F32, tag="a2")
                a3 = small.tile([P, 1], F32, tag="a3")
                nc.scalar.activation(v, u[:, 0:3], Ident, bias=1.0)
                nc.scalar.activation(
                    a2, v[:, 0:1], Ident, scale=v[:, 1:2], bias=gb[:, 0:1]
                )
                nc.scalar.activation(
                    a3, a2, Ident, scale=v[:, 2:3], bias=gb[:, 1:2]
                )
                # diag(a3) on DVE in bf16: (ident*a3)*ident == diag(a3)
                diag = diagp.tile([P, P], CD, tag="diag")
                nc.vector.scalar_tensor_tensor(
                    out=diag, in0=ident_bf, scalar=a3[:, 0:1], in1=ident_bf,
                    op0=mult, op1=mult,
                )

                # out_psum = diag(a3) @ xb + ones2 @ [dsum_hi; dsum_lo]
                ps_out = psum_pool.tile([P, D], F32, tag="ps_out")
                for h in range(2):
                    sl = slice(512 * h, 512 * (h + 1))
                    nc.tensor.matmul(
                        ps_out[:, sl], diag, xb[:, sl], start=True, stop=False
                    )
                for h in range(2):
                    sl = slice(512 * h, 512 * (h + 1))
                    nc.tensor.matmul(
                        ps_out[:, sl], ones2_bf, d16[:, sl], start=False, stop=True
                    )

                ot = op.tile([P, D], F32, tag="ot")
                nc.scalar.copy(ot, ps_out)
                nc.sync.dma_start(out=out[rows, :], in_=ot)

            pending = None
            for i in range(N_TILES * REPEAT):
                state = front(i % N_TILES)
                if pending is not None:
                    back(pending)
                pending = state
            back(pending)

    nc.compile()
    return nc


def _build_balanced() -> Bass:
    # Previous all-fp32 engine split (rel err 8.4e-7, ~2x slower): DVE does
    # u0,u1 in fp32 1x, GPSIMD multiplies for u2, ACT reduces, PE applies
    # diag(a3) in fp32 + bf16 dsum broadcast.
    nc = Bacc("TRN2", target_bir_lowering=False, debug=False, num_devices=N_CORES)
    x = nc.dram_tensor("x", [RPC, D], F32, kind="ExternalInput")
    aux = nc.dram_tensor("aux", [5, D], F32, kind="ExternalInput")
    aux16 = nc.dram_tensor("aux16", [2, D], BF16, kind="ExternalInput")
    out = nc.dram_tensor("out", [RPC, D], F32, kind="ExternalOutput")

    mult = mybir.AluOpType.mult
    add = mybir.AluOpType.add
    Copy = mybir.ActivationFunctionType.Copy

    with TileContext(nc) as tc:
        with (
            tc.tile_pool(name="consts", bufs=1) as consts,
            tc.tile_pool(name="xp", bufs=6) as xp,
            tc.tile_pool(name="scrp", bufs=3) as scrp,
            tc.tile_pool(name="op", bufs=4) as op,
            tc.tile_pool(name="small", bufs=8) as small,
            tc.tile_pool(name="diagp", bufs=3) as diagp,
            tc.tile_pool(name="psum", bufs=2, space="PSUM") as psum_pool,
            tc.tile_pool(name="psum_bc", bufs=2, space="PSUM") as psum_bc,
        ):
            w_row = []
            for t in range(3):
                r = consts.tile([1, D], F32, tag=f"w_row{t}")
                nc.sync.dma_start(out=r, in_=aux[t : t + 1, :])
                w_row.append(r)
            g_row = consts.tile([1, 2], F32, tag="g_row")
            nc.sync.dma_start(out=g_row, in_=aux[4:5, 0:2])
            d16 = consts.tile([2, D], BF16, tag="d16")
            nc.sync.dma_start(out=d16, in_=aux16[:, :])

            ones_col = consts.tile([1, P], F32, tag="ones_col")
            nc.vector.memset(ones_col, 1.0)
            ones2_bf = consts.tile([2, P], BF16, tag="ones2_bf")
            nc.vector.memset(ones2_bf, 1.0)
            ones4 = consts.tile([P, 4], F32, tag="ones4")
            nc.vector.memset(ones4, 1.0)
            ident = consts.tile([P, P], F32, tag="ident")
            make_identity(nc, ident)

            wb = []
            for t in range(3):
                wt = consts.tile([P, D], F32, tag=f"wb{t}")
                for h in range(2):
                    sl = slice(512 * h, 512 * (h + 1))
                    ps = psum_bc.tile([P, 512], F32, tag="bc")
                    nc.tensor.matmul(ps, ones_col, w_row[t][:, sl], start=True, stop=True)
                    nc.scalar.copy(wt[:, sl], ps)
                wb.append(wt)
            gb = consts.tile([P, 2], F32, tag="gb")
            ps = psum_bc.tile([P, 2], F32, tag="bc_g")
            nc.tensor.matmul(ps, ones_col, g_row[:, 0:2], start=True, stop=True)
            nc.scalar.copy(gb, ps)

            for i in range(N_TILES * REPEAT):
                i = i % N_TILES
                rows = slice(i * P, (i + 1) * P)
                xt = xp.tile([P, D], F32, tag="x")
                nc.sync.dma_start(out=xt, in_=x[rows, :])

                u = small.tile([P, 4], F32, tag="u")
                scr = small.tile([P, D], F32, tag="scr")

                scr2 = scrp.tile([P, D], F32, tag="scr2")
                nc.gpsimd.tensor_tensor(scr2, xt, wb[2], op=mult)
                dummy2 = small.tile([P, 1], F32, tag="dummy2")
                nc.scalar.activation(
                    dummy2.broadcast_to((P, D)), scr2, Copy,
                    accum_out=u[:, 2:3],
                )
                for t in range(2):
                    nc.vector.scalar_tensor_tensor(
                        out=scr, in0=xt, scalar=1.0, in1=wb[t],
                        op0=mult, op1=mult, accum_out=u[:, t : t + 1],
                    )

                v = small.tile([P, 3], F32, tag="v")
                a2 = small.tile([P, 1], F32, tag="a2")
                a3 = small.tile([P, 1], F32, tag="a3")
                nc.vector.tensor_tensor(v, u[:, 0:3], ones4[:, 0:3], op=add)
                nc.vector.scalar_tensor_tensor(
                    out=a2, in0=v[:, 0:1], scalar=v[:, 1:2], in1=gb[:, 0:1],
                    op0=mult, op1=add,
                )
                nc.vector.scalar_tensor_tensor(
                    out=a3, in0=a2, scalar=v[:, 2:3], in1=gb[:, 1:2],
                    op0=mult, op1=add,
                )
                diag = diagp.tile([P, P], F32, tag="diag")
                nc.scalar.activation(diag, ident, Copy, scale=a3[:, 0:1])

                ps_out = psum_pool.tile([P, D], F32, tag="ps_out")
                for h in range(2):
                    sl = slice(512 * h, 512 * (h + 1))
                    nc.tensor.matmul(
                        ps_out[:, sl], diag, xt[:, sl], start=True, stop=False
                    )
                for h in range(2):
                    sl = slice(512 * h, 512 * (h + 1))
                    nc.tensor.matmul(
                        ps_out[:, sl], ones2_bf, d16[:, sl], start=False, stop=True
                    )

                ot = op.tile([P, D], F32, tag="ot")
                nc.scalar.copy(ot, ps_out)
                nc.sync.dma_start(out=out[rows, :], in_=ot)

    nc.compile()
    return nc


def _build() -> Bass:
    return _build_bf16() if MODE == "bf16" else _build_balanced()


def _get_program() -> Bass:
    key = f"{MODE}-{LOAD}-{DOT}-{DOTC}-{CDT}-{REPEAT}"
    if key not in _CACHE:
        _CACHE[key] = _build()
    return _CACHE[key]


def _make_aux(weights: np.ndarray, bias: np.ndarray):
    import ml_dtypes

    w = np.asarray(weights, dtype=np.float32)
    b = np.asarray(bias, dtype=np.float32)
    aux = np.zeros((5, D), dtype=np.float32)
    aux[0:3] = w
    dsum = b.sum(axis=0)
    aux[3] = dsum
    aux[4, 0] = float(b[0] @ w[1])
    aux[4, 1] = float((b[0] + b[1]) @ w[2])
    hi = dsum.astype(ml_dtypes.bfloat16)
    lo = (dsum - hi.astype(np.float32)).astype(ml_dtypes.bfloat16)
    aux16 = np.stack([hi, lo])
    return aux, aux16


def kernel(x: np.ndarray, weights: np.ndarray, bias: np.ndarray) -> np.ndarray:
    x = np.ascontiguousarray(np.asarray(x, dtype=np.float32))
    aux, aux16 = _make_aux(weights, bias)
    nc = _get_program()
    in_maps = [
        {"x": x[i * RPC : (i + 1) * RPC], "aux": aux, "aux16": aux16}
        for i in range(N_CORES)
    ]
    res = run_bass_kernel_spmd(nc, in_maps, list(range(N_CORES)), trace=TRACE)
    global LAST_RESULTS
    LAST_RESULTS = res
    return np.concatenate([r["out"] for r in res.results], axis=0)
